# revision 27
# baseline (speedup 1.0000x reference)
"""Bass/Tile kernel for nn_BasicGRUClassifier on 8 Trainium2 NeuronCores.

Strategy (data-parallel over batch, 32 samples/core, bf16 matmul datapath,
cross-layer fusion):

  The two GRU layers are software-pipelined with a fixed lag of LCH=8
  steps and FUSED: one "slot" (c, tl) advances layer0 at t = c*8+tl and
  layer1 at t-8 with SINGLE activation/vector instructions over paired
  [128, 2, 32] / [128, 64] operands. Fusion halves the per-step
  instruction count and semaphore traffic on the serial critical path,
  which is what bounds this latency-dominated recurrence.

  PSUM layout per chunk c (shared banks make the fused APs single-tile):
    RUP(c) [128,1024] (2 banks, bufs=3):
       bank A: r0(c) cols 0:256   | r1(c-1) cols 256:512
       bank B: u0(c) cols 512:768 | u1(c-1) cols 768:1024
    OP(c)  [128,512] (1 bank, bufs=2): o0(c) 0:256 | o1(c-1) 256:512
  Banks are seeded by batched x-projection matmuls (L0 biases ride a
  ones-channel appended to X's last K-tile; L1 biases are K=1 matmuls
  against a ones row).

  The state update h' = (1-u)h + u*o is decomposed as m = (u-1)h,
  e = u*o, and the next step's gate pre-activations are accumulated as
  x + (-U)@m + U@e directly in PSUM (pre-negated weight copies), so the
  only work between tanh and the next step's matmuls is one vector op
  (e). h' = e - m itself is computed off the critical path (GpSimd) into
  paired state tiles: hpair(c) slot tl holds [h0(t) | h1(t-8)], written
  by one op and consumed as one operand by the next slot.

  Everything the PE touches is bf16 (fp32 matmuls double-pump the PE);
  PSUM accumulation stays fp32; activations read fp32 PSUM and emit
  bf16. bf16 end-to-end rel err vs fp32 reference = 4.1e-3 (tol 2e-2).
"""

import numpy as np
import ml_dtypes

HID = 128
IN_CH = 271
SEQ = 281
NCLS = 1854
BATCH = 256
NCORES = 8
BL = BATCH // NCORES  # 32 per-core batch
LCH = 8               # timesteps per chunk == layer pipeline lag
G3 = 3 * HID
CW = LCH * BL         # 256: one gate region width
R0, R1, U0, U1 = 0, CW, 2 * CW, 3 * CW
O0, O1 = 0, CW

_CACHE = {}


def _build(seq_t):
    import concourse.bacc as bacc
    import concourse.tile as tile
    import concourse.mybir as mybir
    from contextlib import ExitStack

    fp32 = mybir.dt.float32
    bf16 = mybir.dt.bfloat16
    AF = mybir.ActivationFunctionType
    ALU = mybir.AluOpType

    nch = (seq_t + LCH - 1) // LCH
    chlen = [min(LCH, seq_t - c * LCH) for c in range(nch)]

    nc = bacc.Bacc()
    XT = nc.dram_tensor("XT", [IN_CH, seq_t * BL], bf16, kind="ExternalInput")
    WX0 = nc.dram_tensor("WX0", [IN_CH + 1, G3], bf16, kind="ExternalInput")
    UH0 = nc.dram_tensor("UH0", [HID, G3], bf16, kind="ExternalInput")
    UN0 = nc.dram_tensor("UN0", [HID, 2 * HID], bf16, kind="ExternalInput")
    WX1 = nc.dram_tensor("WX1", [HID, G3], bf16, kind="ExternalInput")
    UH1 = nc.dram_tensor("UH1", [HID, G3], bf16, kind="ExternalInput")
    UN1 = nc.dram_tensor("UN1", [HID, 2 * HID], bf16, kind="ExternalInput")
    B1R = nc.dram_tensor("B1R", [1, G3], bf16, kind="ExternalInput")
    WFC = nc.dram_tensor("WFC", [HID, NCLS], bf16, kind="ExternalInput")
    BFC = nc.dram_tensor("BFC", [1, NCLS], bf16, kind="ExternalInput")
    OUT = nc.dram_tensor("OUT", [BL, NCLS], fp32, kind="ExternalOutput")

    ksz = [128, 128, IN_CH - 256 + 1]  # third tile: 15 channels + ones row

    with tile.TileContext(nc) as tc:
        with ExitStack() as ctx:
            const = ctx.enter_context(tc.tile_pool(name="const", bufs=1))
            hps = ctx.enter_context(tc.tile_pool(name="hps", bufs=3))
            cellp = ctx.enter_context(tc.tile_pool(name="cellp", bufs=8))
            outp = ctx.enter_context(tc.tile_pool(name="outp", bufs=1))
            rup = ctx.enter_context(tc.tile_pool(name="rup", bufs=3, space="PSUM"))
            opp = ctx.enter_context(tc.tile_pool(name="opp", bufs=2, space="PSUM"))

            # ---- constants into SBUF ----
            xt_sb = []
            for k in range(3):
                t_ = const.tile([ksz[k], seq_t * BL], bf16, tag=f"xt{k}")
                c0 = sum(ksz[:k])
                if k < 2:
                    nc.sync.dma_start(out=t_, in_=XT[c0:c0 + ksz[k], :])
                else:
                    # row 15 is the ones-channel carrying the L0 biases
                    nc.vector.memset(t_, 1.0)
                    nc.sync.dma_start(out=t_[0:15, :], in_=XT[256:271, :])
                xt_sb.append(t_)
            wx0_sb = []
            for k in range(3):
                t_ = const.tile([ksz[k], G3], bf16, tag=f"wx0{k}")
                c0 = sum(ksz[:k])
                nc.sync.dma_start(out=t_, in_=WX0[c0:c0 + ksz[k], :])
                wx0_sb.append(t_)

            def sbconst(name, dram, shape, dt=bf16):
                t_ = const.tile(shape, dt, tag=name)
                nc.sync.dma_start(out=t_, in_=dram[:, :])
                return t_

            uh0_sb = sbconst("uh0", UH0, [HID, G3])
            un0_sb = sbconst("un0", UN0, [HID, 2 * HID])
            wx1_sb = sbconst("wx1", WX1, [HID, G3])
            uh1_sb = sbconst("uh1", UH1, [HID, G3])
            un1_sb = sbconst("un1", UN1, [HID, 2 * HID])
            b1_sb = sbconst("b1", B1R, [1, G3])
            wfc_sb = sbconst("wfc", WFC, [HID, NCLS])
            bfc_sb = sbconst("bfc", BFC, [1, NCLS])
            ones_sb = const.tile([1, CW], bf16, tag="ones")
            nc.vector.memset(ones_sb, 1.0)
            h0i = const.tile([HID, BL], bf16, tag="h0i")
            nc.vector.memset(h0i, 0.0)

            rupt = {}
            opt_ = {}
            hpair = {}

            pending = []

            def drain_pending(k=2):
                for _ in range(min(k, len(pending))):
                    pending.pop(0)()

            def phase_l0(c):
                """Chunk c's L0 r/u/o x-projections, split into N=128 pieces
                so a queued piece never blocks a critical cell matmul for
                long, plus the L1 biases for chunk c-1 (whose gates share
                these banks). k==0/h==0 start=True clears each bank and must
                precede every other write to it."""
                n = chlen[c] * BL
                t0 = c * LCH * BL
                ru = rup.tile([HID, 4 * CW], fp32, tag="rup")
                ob = opp.tile([HID, 2 * CW], fp32, tag="opp")
                rupt[c] = ru
                opt_[c] = ob
                nh = (n + 127) // 128
                for g, dst in ((0, ru), (1, ru), (2, ob)):
                    off = (R0, U0, O0)[g]
                    for k in range(3):
                        for h in range(nh):
                            a, bnd = h * 128, min(n, (h + 1) * 128)
                            def mm(g=g, dst=dst, off=off, k=k, a=a, bnd=bnd):
                                nc.tensor.matmul(
                                    dst[:, off + a:off + bnd],
                                    wx0_sb[k][:, g * HID:(g + 1) * HID],
                                    xt_sb[k][:, t0 + a:t0 + bnd],
                                    start=(k == 0 and a == 0), stop=False)
                            pending.append(mm)
                if c > 0:
                    phase_l1_bias(c - 1, False)

            def phase_l1_bias(c1, first):
                """Bias seed for L1 chunk c1 (gates live in chunk c1+1's
                banks). With first=True (past the last L0 chunk) the target
                banks are fresh: allocate and let the bias matmuls clear."""
                n = chlen[c1] * BL
                if first:
                    ru_n = rup.tile([HID, 4 * CW], fp32, tag="rup")
                    ob_n = opp.tile([HID, 2 * CW], fp32, tag="opp")
                    rupt[c1 + 1] = ru_n
                    opt_[c1 + 1] = ob_n
                ru = rupt[c1 + 1]
                ob = opt_[c1 + 1]
                for off, wslice, dst in (
                        (R1, (0, HID), ru), (U1, (HID, 2 * HID), ru),
                        (O1, (2 * HID, G3), ob)):
                    def mmb(off=off, wslice=wslice, dst=dst):
                        nc.tensor.matmul(
                            dst[:, off:off + n], b1_sb[:, wslice[0]:wslice[1]],
                            ones_sb[:, 0:n], start=first, stop=False)
                    pending.append(mmb)

            def emit_next_mms(parts, which):
                """Accumulate (-U)@m (which=0, stop=False) or U@e (which=1,
                stop=True) into the next step's gate columns. parts is a list
                of (uh, un, rhs, t_next, roff, uoff); the r-gate matmuls of
                all layers are emitted before the u-gate ones so the next
                sigma(r) waits on as few matmuls as possible."""
                for gsel in (0, 1):
                    for uh, un, rhs, t_next, roff, uoff in parts:
                        cn, sn = divmod(t_next, LCH)
                        ru = rupt[cn] if roff == R0 else rupt[cn + 1]
                        off = (roff, uoff)[gsel]
                        s = sn * BL
                        w = (un, uh)[which]
                        nc.tensor.matmul(
                            ru[:, off + s:off + s + BL],
                            w[:, gsel * HID:(gsel + 1) * HID], rhs,
                            start=False, stop=(which == 1))

            def slot(c, tl, fused, l0_only, hp_prev, h_out):
                """One pipeline slot: layer0 at t=c*8+tl (unless tail),
                layer1 at t-8 (if fused or tail). Operand width W is 64 for
                fused slots, 32 otherwise.

                hp_prev: [128, W] AP of the previous state (pair); h_out:
                [128, W] AP to write the new state (pair)."""
                t = c * LCH + tl
                s = tl * BL
                W = 2 * BL if fused else BL
                nl = 2 if fused else 1
                ru = rupt[c]
                ob = opt_[c]
                if fused:
                    rsrc = ru.rearrange("p (g x) -> p g x", g=4)[
                        :, 0:2, s:s + BL]
                    usrc = ru.rearrange("p (g x) -> p g x", g=4)[
                        :, 2:4, s:s + BL]
                    osrc = ob.rearrange("p (g x) -> p g x", g=2)[
                        :, :, s:s + BL]
                elif l0_only:
                    rsrc = ru[:, R0 + s:R0 + s + BL]
                    usrc = ru[:, U0 + s:U0 + s + BL]
                    osrc = ob[:, O0 + s:O0 + s + BL]
                else:  # tail: layer1 only; gates live in chunk c+1's banks
                    ru = rupt[c + 1]
                    ob = opt_[c + 1]
                    rsrc = ru[:, R1 + s:R1 + s + BL]
                    usrc = ru[:, U1 + s:U1 + s + BL]
                    osrc = ob[:, O1 + s:O1 + s + BL]

                ru_r = cellp.tile([HID, W], bf16, tag="rur")
                ru_u = cellp.tile([HID, W], bf16, tag="ruu")
                if fused:
                    nc.scalar.activation(
                        ru_r.rearrange("p (l x) -> p l x", l=2), rsrc,
                        AF.Sigmoid)
                    nc.scalar.activation(
                        ru_u.rearrange("p (l x) -> p l x", l=2), usrc,
                        AF.Sigmoid)
                else:
                    nc.scalar.activation(ru_r[:, 0:BL], rsrc, AF.Sigmoid)
                    nc.scalar.activation(ru_u[:, 0:BL], usrc, AF.Sigmoid)
                rh = cellp.tile([HID, W], bf16, tag="rh")
                nc.vector.tensor_mul(rh[:, 0:W], ru_r[:, 0:W], hp_prev)
                m = cellp.tile([HID, W], bf16, tag="m")
                nc.vector.scalar_tensor_tensor(
                    m[:, 0:W], ru_u[:, 0:W], 1.0, hp_prev,
                    op0=ALU.subtract, op1=ALU.mult)

                def parts_for(x):
                    ps = []
                    if (l0_only or fused) and t + 1 < seq_t:
                        ps.append((uh0_sb, un0_sb, x[:, 0:BL], t + 1, R0, U0))
                    if not l0_only:
                        t1 = t - LCH if fused else t
                        xo = x[:, BL:2 * BL] if fused else x[:, 0:BL]
                        if t1 + 1 < seq_t:
                            ps.append((uh1_sb, un1_sb, xo, t1 + 1, R1, U1))
                    return ps

                # (-U)@m accumulation into the next step's gate columns
                emit_next_mms(parts_for(m), 0)
                # o-gate matmuls
                if l0_only or fused:
                    nc.tensor.matmul(ob[:, O0 + s:O0 + s + BL],
                                     uh0_sb[:, 2 * HID:G3], rh[:, 0:BL],
                                     start=False, stop=True)
                if not l0_only:
                    rho = rh[:, BL:2 * BL] if fused else rh[:, 0:BL]
                    nc.tensor.matmul(ob[:, O1 + s:O1 + s + BL],
                                     uh1_sb[:, 2 * HID:G3], rho,
                                     start=False, stop=True)
                # a deferred phase matmul here executes in the tanh shadow
                drain_pending(1)
                o_t = cellp.tile([HID, W], bf16, tag="ot")
                if fused:
                    nc.scalar.activation(
                        o_t.rearrange("p (l x) -> p l x", l=2), osrc, AF.Tanh)
                else:
                    nc.scalar.activation(o_t[:, 0:BL], osrc, AF.Tanh)
                e = cellp.tile([HID, W], bf16, tag="e")
                nc.vector.tensor_mul(e[:, 0:W], ru_u[:, 0:W], o_t[:, 0:W])
                # U@e accumulation into the next step's gate columns
                emit_next_mms(parts_for(e), 1)
                # h' = e - m, off the critical path
                nc.gpsimd.tensor_sub(h_out, e[:, 0:W], m[:, 0:W])
                # incremental L1 x-projection: Wx1 @ h0(t) feeds layer1's
                # step t, which runs LCH slots from now -- far off the
                # critical path, and it removes the batched end-of-chunk
                # projection burst that used to sit right in front of the
                # next chunk's first sigma
                if l0_only or fused:
                    cn1 = t // LCH + 1
                    s1 = (t % LCH) * BL
                    h0new = h_out[:, 0:BL]
                    last = t == 0  # t'=0 gets no m/e matmuls: close its group
                    nc.tensor.matmul(
                        rupt[cn1][:, R1 + s1:R1 + s1 + BL], wx1_sb[:, 0:HID],
                        h0new, start=False, stop=last)
                    nc.tensor.matmul(
                        rupt[cn1][:, U1 + s1:U1 + s1 + BL],
                        wx1_sb[:, HID:2 * HID], h0new, start=False, stop=last)
                    nc.tensor.matmul(
                        opt_[cn1][:, O1 + s1:O1 + s1 + BL],
                        wx1_sb[:, 2 * HID:G3], h0new, start=False, stop=False)
                # end-of-slot deferred phase matmuls: they execute during the
                # next slot's sigma/rh/m window, never between a slot's
                # critical matmul groups
                drain_pending(2)
                return h_out

            # ---------- main pipeline ----------
            phase_l0(0)
            phase_l0(1)

            for c in range(nch):
                hp = hps.tile([HID, LCH * 2 * BL], bf16, tag="hpair")
                hpair[c] = hp
                if c == 0:
                    nc.vector.memset(hp, 0.0)
                for tl in range(chlen[c]):
                    if tl == 0:
                        # flush everything queued so far: this chunk's banks
                        # (phase_l0(c) + the L1 biases) must be emitted before
                        # the sigmas and incremental projections that use them
                        drain_pending(len(pending))
                        if c + 2 < nch:
                            phase_l0(c + 2)
                        elif c + 2 == nch:
                            # chunk nch's banks hold only the tail L1 chunk's
                            # gates: bias matmuls allocate and clear them
                            phase_l1_bias(nch - 1, True)
                    t = c * LCH + tl
                    if tl == 0:
                        hp_prev_t = hpair[c - 1] if c > 0 else None
                        pslot = (LCH - 1) * 2 * BL
                    else:
                        hp_prev_t = hp
                        pslot = (tl - 1) * 2 * BL
                    if c == 0:
                        hp_prev = h0i[:, 0:BL] if tl == 0 else \
                            hp_prev_t[:, pslot:pslot + BL]
                        h_out = hp[:, tl * 2 * BL:tl * 2 * BL + BL]
                        slot(c, tl, False, True, hp_prev, h_out)
                    else:
                        hp_prev = hp_prev_t[:, pslot:pslot + 2 * BL]
                        h_out = hp[:, tl * 2 * BL:(tl + 1) * 2 * BL]
                        slot(c, tl, True, False, hp_prev, h_out)

            # ---------- tail: remaining layer1 steps ----------
            # fused slots covered layer1 through t1 = seq_t-1-LCH; the last
            # LCH steps run unfused. h1(t1-1) was written by the fused slot
            # pairing layer0 step t1-1+LCH.
            tp = seq_t - LCH - 1 + LCH  # = seq_t-1: slot of h1(seq_t-LCH-1)
            cp, tlp = divmod(tp, LCH)
            h1_cur = hpair[cp][:, tlp * 2 * BL + BL:(tlp + 1) * 2 * BL]
            for t1 in range(seq_t - LCH, seq_t):
                c1, tl1 = divmod(t1, LCH)
                if tl1 == 0:
                    drain_pending(len(pending))
                h1n = cellp.tile([HID, BL], bf16, tag="h1t")
                slot(c1, tl1, False, False, h1_cur, h1n[:, :])
                h1_cur = h1n[:, :]

            drain_pending(len(pending))

            # ---------- FC ----------
            out_sb = outp.tile([BL, NCLS], fp32, tag="osb")
            nsl = [512, 512, 512, NCLS - 3 * 512]
            for i in range(4):
                n0 = i * 512
                fc = rup.tile([BL, 512], fp32, tag="rup")
                pf = fc[:, 0:nsl[i]]
                nc.tensor.matmul(pf, ones_sb[:, 0:BL], bfc_sb[:, n0:n0 + nsl[i]],
                                 start=True, stop=False)
                nc.tensor.matmul(pf, h1_cur, wfc_sb[:, n0:n0 + nsl[i]],
                                 start=False, stop=True)
                nc.scalar.activation(out_sb[:, n0:n0 + nsl[i]], pf, AF.Identity)
            nc.sync.dma_start(out=OUT[:, :], in_=out_sb)

    nc.finalize()
    return nc


def _prep_consts(inputs):
    bf = ml_dtypes.bfloat16
    Wx0 = np.ascontiguousarray(np.concatenate([
        np.concatenate([inputs["Wr0"][:IN_CH], inputs["Wu0"][:IN_CH],
                        inputs["Wo0"][:IN_CH]], axis=1),
        np.concatenate([inputs["br0"], inputs["bu0"], inputs["bo0"]])[None, :],
    ], axis=0).astype(bf))
    Uh0 = np.concatenate(
        [inputs["Wr0"][IN_CH:], inputs["Wu0"][IN_CH:], inputs["Wo0"][IN_CH:]],
        axis=1).astype(bf)
    Uh1 = np.concatenate(
        [inputs["Wr1"][HID:], inputs["Wu1"][HID:], inputs["Wo1"][HID:]],
        axis=1).astype(bf)
    Un0 = np.ascontiguousarray(-Uh0[:, 0:2 * HID])
    Un1 = np.ascontiguousarray(-Uh1[:, 0:2 * HID])
    Wx1 = np.ascontiguousarray(np.concatenate(
        [inputs["Wr1"][:HID], inputs["Wu1"][:HID], inputs["Wo1"][:HID]],
        axis=1).astype(bf))
    B1R = np.ascontiguousarray(np.concatenate(
        [inputs["br1"], inputs["bu1"], inputs["bo1"]])[None, :].astype(bf))
    WFC = np.ascontiguousarray(inputs["Wfc"].astype(bf))
    BFC = np.ascontiguousarray(inputs["bfc"][None, :].astype(bf))
    return dict(WX0=Wx0, UH0=np.ascontiguousarray(Uh0), UN0=Un0,
                WX1=Wx1, UH1=np.ascontiguousarray(Uh1), UN1=Un1,
                B1R=B1R, WFC=WFC, BFC=BFC)


def kernel(_trace=False, **inputs):
    from concourse.bass_utils import run_bass_kernel_spmd

    seq_t = inputs["X"].shape[2]
    if "nc" not in _CACHE or _CACHE.get("seq_t") != seq_t:
        _CACHE["nc"] = _build(seq_t)
        _CACHE["seq_t"] = seq_t
    nc = _CACHE["nc"]

    consts = _prep_consts(inputs)
    bf = ml_dtypes.bfloat16
    # [B, C, T] -> per-core [C, T, BL] (t-major columns: col = t*BL + b)
    X = inputs["X"].astype(bf)
    in_maps = []
    for c in range(NCORES):
        m = dict(consts)
        xc = X[c * BL:(c + 1) * BL].transpose(1, 2, 0)  # [C, T, BL]
        m["XT"] = np.ascontiguousarray(xc).reshape(IN_CH, seq_t * BL)
        in_maps.append(m)

    res = run_bass_kernel_spmd(nc, in_maps, core_ids=list(range(NCORES)),
                               trace=_trace)
    out = np.concatenate([r["OUT"] for r in res.results], axis=0)
    if _trace:
        _CACHE["last_exec_time_ns"] = res.exec_time_ns
        _CACHE["last_profile"] = res.profile_json
    return out
